# revision 29
# baseline (speedup 1.0000x reference)
"""Trainium2 Bass kernel for CustomTaylorLayer.

Computes out[b, j] = sum_{i,k} coef[j, i, k] * tanh(x[b, i] * r)^k
for x:[8192,1024], coef:[1024,1024,8], r scalar.

Strategy: data-parallel over the batch across 8 NeuronCores (1024 rows
per core). Mixed precision chosen from a host-side error study against
the 2e-2 gate (measured on the real data distribution):
  - k=1..5 matmuls in bf16 (separate LDWEIGHTS + FWL fully hides the
    weight load under the N=512 moving stream -> ~216 ns/MM pace vs
    ~288 ns for f32r whose 4-byte weights self-load inside the MM),
  - k=6,7 as fp8(e4m3) DoubleRow matmuls: both slabs packed into one
    256-row virtual contraction at 2 rows/cycle,
  - k=0 reduced to per-j column sums on the host (exact, fp64) and
    folded in during the PSUM drain as a per-partition scalar add.
Measured model error bf16+fp8{6,7} ~= 1.2e-2 < 2e-2.

All 7 k-terms accumulate in PSUM (one bank per (j-tile, batch-half);
4 j-tiles x 2 halves = 8 banks per j-group pass, 2 passes) so the
vector engine does no inter-k adds at all -- only the tanh-power
recurrence and the final drain. Weights stream per (j-group, k) slab
in host-pre-blocked contiguous 128KB chunks, so total weight traffic
is the minimal 10.5 MB. Dummy warmup matmuls hold the PE HAM clock
gate at 2.4 GHz through the startup DMA phase.
"""

import numpy as np
from contextlib import ExitStack

B, IN, OUT, K = 8192, 1024, 1024, 8
NCORES = 8
BLOC = B // NCORES          # 1024 batch rows per core
NI = IN // 128              # 8 i-tiles (contraction)
NJ = OUT // 128             # 8 j-tiles (output)
NH = BLOC // 512            # 2 moving-dim halves (PSUM bank = 512 fp32)
JGS = 4                     # j-tiles per group (4 x 2 halves = 8 PSUM banks)
NJG = NJ // JGS             # 2 j-group passes
NKB = 5                     # bf16 k-slabs: k = 1..5

_NC_CACHE = {}


def _build_nc():
    import concourse.bacc as bacc
    import concourse.mybir as mybir
    import concourse.tile as tile

    dt = mybir.dt
    AF = mybir.ActivationFunctionType
    f32 = dt.float32
    bf16 = dt.bfloat16
    f8 = dt.float8e4
    DR = mybir.MatmulPerfMode.DoubleRow

    nc = bacc.Bacc("TRN2", target_bir_lowering=False, debug=False)

    # ii=0 pre-split in halves (earliest tanh); ii>=1 as full 256KB chunks
    # whose 2KB-per-partition lines get full DMA efficiency.
    xt0_d = nc.dram_tensor(
        "xt0", [NH, 128, 512], bf16, kind="ExternalInput").ap()
    xtr_d = nc.dram_tensor(
        "xtr", [NI - 1, 128, BLOC], bf16, kind="ExternalInput").ap()
    wb_d = nc.dram_tensor(
        "wb", [NJG, NKB, NI, 128, JGS * 128], bf16, kind="ExternalInput").ap()
    w67_d = nc.dram_tensor(
        "w67", [NJG, NI, 128, JGS, 2, 128], f8, kind="ExternalInput").ap()
    rng_d = nc.dram_tensor("rng", [128, 1], f32, kind="ExternalInput").ap()
    scol_d = nc.dram_tensor("scol", [128, NJ], f32, kind="ExternalInput").ap()
    out_d = nc.dram_tensor("outT", [OUT, BLOC], f32, kind="ExternalOutput").ap()

    with tile.TileContext(nc) as tc, ExitStack() as ctx:
        sb = ctx.enter_context(tc.tile_pool(name="sb", bufs=1))
        wp = ctx.enter_context(tc.tile_pool(name="wp", bufs=3))
        w67p = ctx.enter_context(tc.tile_pool(name="w67p", bufs=2))
        xp = ctx.enter_context(tc.tile_pool(name="xp", bufs=8))
        op = ctx.enter_context(tc.tile_pool(name="op", bufs=4))
        pp = ctx.enter_context(tc.tile_pool(name="pp", bufs=8, space="PSUM"))

        # Sync-queue order drives the critical path to the first real MM:
        # first xt half-chunk, then r (host-prebroadcast [128,1], 512B),
        # then the rest of xt. s_cols rides last -- its first use is the
        # first drain at ~90us.
        xss = []
        xs0 = xp.tile([128, 1, BLOC], bf16, tag="x", name="xs0")
        nc.sync.dma_start(xs0[:, 0, 0:512], xt0_d[0])
        r_col = sb.tile([128, 1], f32, tag="rcol")
        nc.sync.dma_start(r_col[:], rng_d)
        nc.sync.dma_start(xs0[:, 0, 512:1024], xt0_d[1])
        xss.append(xs0)
        for it in range(1, NI):
            xs = xp.tile([128, 1, BLOC], bf16, tag="x", name=f"xs{it}")
            nc.sync.dma_start(xs[:, 0, :], xtr_d[it - 1])
            xss.append(xs)
        s_cols = sb.tile([128, NJ], f32, tag="scol")
        nc.sync.dma_start(s_cols[:], scol_d)

        ones = sb.tile([128, 512], bf16, tag="ones")
        nc.vector.memset(ones[:], 1.0)

        # Early DMAs: first xt chunk + jg0/k1 weight slab feed the first
        # real matmuls; weights ride the GpSimd (SWDGE) queues.
        def load_w(jg, kk):
            wk = wp.tile([128, NI, JGS * 128], bf16, tag="w")
            for ii in range(NI):
                nc.gpsimd.dma_start(wk[:, ii, :], wb_d[jg, kk, ii])
            return wk

        def load_w67(jg):
            wt = w67p.tile([128, NI, JGS, 2, 128], f8, tag="w67")
            for ii in range(NI):
                nc.gpsimd.dma_start(wt[:, ii], w67_d[jg, ii])
            return wt

        # Warm the PE HAM clock gate during the startup DMA phase so real
        # MMs run at 2.4 GHz (~3.4us of sustained PE activity needed).
        wps = pp.tile([128, 512], f32, tag="ps", bufs=8)
        for wv in range(11):
            nc.tensor.matmul(wps[:], ones[:, 0:128], ones[:, 0:512],
                             start=(wv == 0), stop=(wv == 10))

        # Phase 1: t1 = tanh(xT * r). The ACT queue paces k=1, so ii=0 runs
        # in halves (earliest possible first matmul) and the rest as full
        # chunks (fewer per-instruction overheads on the serial ACT train).
        t1 = sb.tile([128, NI, BLOC], bf16, tag="t1")
        for h in range(NH):
            sl = slice(h * 512, (h + 1) * 512)
            nc.scalar.activation(
                t1[:, 0, sl], xss[0][:, 0, sl], AF.Tanh,
                scale=r_col[:, 0:1])
        for it in range(1, NI):
            nc.scalar.activation(
                t1[:, it, :], xss[it][:, 0, :], AF.Tanh,
                scale=r_col[:, 0:1])

        # Power recurrence on DVE (bf16 -> 2x rate): t_k = t_{k-1} * t1.
        # t6/t7 additionally land as fp8 pairs for the DoubleRow matmuls:
        # t67[:, ii, s, :] = t^(6+s) in e4m3.
        tks = [t1]
        for k in range(2, 7):
            tk = sb.tile([128, NI, BLOC], bf16, tag=f"t{k}")
            for it in range(NI):
                nc.vector.tensor_mul(tk[:, it, :], tks[-1][:, it, :],
                                     t1[:, it, :])
            tks.append(tk)
        t6 = tks[5]
        t67 = sb.tile([128, NI, 2, BLOC], f8, tag="t67")
        for it in range(NI):
            nc.vector.tensor_copy(t67[:, it, 0, :], t6[:, it, :])
        for it in range(NI):
            nc.vector.tensor_mul(t67[:, it, 1, :], t6[:, it, :],
                                 t1[:, it, :])

        # Main loop: per j-group, accumulate all k in 8 PSUM banks
        # (4 j-tiles x 2 batch halves), then drain once with the k=0
        # column-sum term folded in as a per-partition scalar add.
        for jg in range(NJG):
            ps = [[pp.tile([128, 512], f32, tag="ps", bufs=8,
                           name=f"ps_{jg}_{j}_{h}")
                   for h in range(NH)] for j in range(JGS)]
            for kk in range(NKB):            # k = kk + 1
                wk = load_w(jg, kk)
                if kk == 1:
                    # w67 isn't consumed until after k=5; keep its DMAs
                    # behind the startup-critical k=1/k=2 slabs.
                    w67t = load_w67(jg)
                src = tks[kk]
                for ii in range(NI):
                    for j in range(JGS):
                        wt = wk[:, ii, j * 128:(j + 1) * 128]
                        for h in range(NH):
                            nc.tensor.matmul(
                                ps[j][h][:],
                                wt,
                                src[:, ii, h * 512:(h + 1) * 512],
                                start=(kk == 0 and ii == 0), stop=False)
            # DoubleRow k=6,7 with j outermost: each j's accumulation group
            # stops early, so its drain + output DMA overlap the remaining
            # matmuls instead of serializing after the last one.
            for j in range(JGS):
                jt = jg * JGS + j
                for ii in range(NI):
                    for h in range(NH):
                        nc.tensor.matmul(
                            ps[j][h][:],
                            w67t[:, ii, j],
                            t67[:, ii, :, h * 512:(h + 1) * 512],
                            start=False, stop=(ii == NI - 1),
                            perf_mode=DR)
                for h in range(NH):
                    # Drain h=0 on DVE, h=1 on ScalarE (Identity act with
                    # per-partition bias = k=0 colsum) so both halves drain
                    # concurrently and the final tail is one drain deep.
                    ob = op.tile([128, 512], f32, tag="o", name=f"ob{jt}{h}")
                    if h == 0:
                        nc.vector.tensor_scalar_add(
                            ob[:], ps[j][h][:], s_cols[:, jt:jt + 1])
                    else:
                        nc.scalar.activation(
                            ob[:], ps[j][h][:], AF.Identity,
                            bias=s_cols[:, jt:jt + 1])
                    # All outputs ride the Sync queue: SBUF->DRAM via the
                    # GpSimd SWDGE path measured a ~9.6us completion drain
                    # in teardown. The very last tile goes out in partition
                    # halves so its two transfers pipeline on the ring.
                    if jg == NJG - 1 and j == JGS - 1 and h == NH - 1:
                        for p0 in (0, 64):
                            nc.sync.dma_start(
                                out_d[jt * 128 + p0:jt * 128 + p0 + 64,
                                      h * 512:(h + 1) * 512],
                                ob[p0:p0 + 64, :])
                    else:
                        nc.sync.dma_start(
                            out_d[jt * 128:(jt + 1) * 128,
                                  h * 512:(h + 1) * 512], ob[:])

    nc.compile()
    return nc


def _get_nc():
    if "nc" not in _NC_CACHE:
        _NC_CACHE["nc"] = _build_nc()
    return _NC_CACHE["nc"]


def _make_in_maps(x, tanh_range, coef):
    import ml_dtypes

    bf16 = ml_dtypes.bfloat16
    f8 = ml_dtypes.float8_e4m3

    x = np.asarray(x, dtype=np.float32)
    coef = np.asarray(coef, dtype=np.float32)
    rng = np.broadcast_to(
        np.asarray(tanh_range, dtype=np.float32).reshape(1, 1),
        (128, 1)).copy()

    # bf16 slabs k=1..5, blocked [jg, kk, ii, p, (j c)] so every DMA is a
    # contiguous 128KB block with partition-major layout.
    wb = coef[:, :, 1:1 + NKB].reshape(NJG, JGS, 128, NI, 128, NKB)
    wb = np.ascontiguousarray(wb.transpose(0, 5, 3, 4, 1, 2)).reshape(
        NJG, NKB, NI, 128, JGS * 128).astype(bf16)

    # fp8 DoubleRow pairs for k=6,7: [jg, ii, p, j, s, c].
    w67 = coef[:, :, 6:8].reshape(NJG, JGS, 128, NI, 128, 2)
    w67 = np.ascontiguousarray(
        w67.transpose(0, 3, 4, 1, 5, 2)).astype(f8)

    # k=0 term: exact column sums, laid out [p, jt].
    s = coef[:, :, 0].astype(np.float64).sum(axis=1)
    scol = np.ascontiguousarray(
        s.reshape(NJ, 128).T).astype(np.float32)

    in_maps = []
    for c in range(NCORES):
        xt = np.ascontiguousarray(
            x[c * BLOC:(c + 1) * BLOC, :].T).astype(bf16)
        xt0 = np.ascontiguousarray(
            xt[0:128].reshape(128, NH, 512).transpose(1, 0, 2))
        xtr = xt[128:].reshape(NI - 1, 128, BLOC)
        in_maps.append({"xt0": xt0, "xtr": xtr, "wb": wb, "w67": w67,
                        "rng": rng, "scol": scol})
    return in_maps


def _ensure_ntff_hook():
    """Register the axon NTFF profile hook if the image's antenv lacks it."""
    import sys
    import types
    try:
        from antenv.axon_hooks import get_axon_ntff_profile_hook  # noqa: F401
        return
    except ImportError:
        pass
    try:
        from trn_agent_boot.trn_boot import _ntff_profile_via_ctypes
        hook = _ntff_profile_via_ctypes("/opt/axon/libaxon_pjrt.so")
    except Exception:
        hook = None
    mod = types.ModuleType("antenv.axon_hooks")
    state = {"hook": hook}
    mod.set_axon_ntff_profile_hook = lambda h: state.__setitem__("hook", h)
    mod.get_axon_ntff_profile_hook = lambda: state["hook"]
    sys.modules["antenv.axon_hooks"] = mod
    import antenv
    antenv.axon_hooks = mod


def _run(x, tanh_range, coef, trace=False):
    from concourse.bass_utils import run_bass_kernel_spmd

    if trace:
        _ensure_ntff_hook()

    nc = _get_nc()
    in_maps = _make_in_maps(x, tanh_range, coef)
    res = run_bass_kernel_spmd(nc, in_maps, core_ids=list(range(NCORES)),
                               trace=trace)
    out = np.empty((B, OUT), dtype=np.float32)
    for c in range(NCORES):
        out[c * BLOC:(c + 1) * BLOC, :] = res.results[c]["outT"].T
    return out, res


def kernel(x, tanh_range, coef):
    out, _ = _run(x, tanh_range, coef, trace=False)
    return out
